# revision 13
# baseline (speedup 1.0000x reference)
"""Self-contained Trainium2 kernel for nn_MultiHeadAttention_91070486544496.

B=4, S=2048, D=1024, H=16 causal MHA. 8-core SPMD: head-parallel
QKV+attention (2 heads/core), mid-attention AllToAll reshard, then
position-parallel output projection.

v4: bf16 matmul pipeline (fp32 streams at 2 cyc/col on the PE array);
causal diagonal via partial-width score MMs plus a 128-periodic
triangle-mask band tile (one start=True per PSUM bank per group —
start poisons the whole 2KB bank); paired [128,1024] score tiles to
amortize ACT op overhead; a global rolling ctx-MM queue that crosses
stage boundaries so the in-order PE queue never waits on the current
stage's last exp; softmax normalization (ln/exp on ACT) phase-split
across stages; host pre-laid per-partition-contiguous DRAM inputs;
DMA dispatch spread across sync+gpsimd queues; next-batch QKV
interleaved into the attention stream as PE filler.
"""
import sys

for _p in ("/opt/trn_rl_repo", "/root/.axon_site/_ro/trn_rl_repo"):
    if _p not in sys.path:
        sys.path.append(_p)

import numpy as np

# ======== runtime infra (axon NTFF hook, BIR wait splitter) ========

import contextlib
import ctypes
import json
import sys
import types

_SO_PATH = "/opt/axon/libaxon_pjrt.so"


def _ntff_profile_via_ctypes(so_path):
    lib = ctypes.CDLL(so_path)
    if not hasattr(lib, "axon_start_nrt_profile"):
        return None
    lib.axon_start_nrt_profile.argtypes = [
        ctypes.POINTER(ctypes.c_int64),
        ctypes.c_size_t,
    ]
    lib.axon_start_nrt_profile.restype = ctypes.c_int64
    lib.axon_stop_nrt_profile.argtypes = [ctypes.c_char_p]
    lib.axon_stop_nrt_profile.restype = ctypes.c_int64

    @contextlib.contextmanager
    def _hook(output_dir, device_ids):
        import jax
        jax.devices()
        if device_ids:
            ids = (ctypes.c_int64 * len(device_ids))(*device_ids)
            rc = lib.axon_start_nrt_profile(ids, len(device_ids))
        else:
            rc = lib.axon_start_nrt_profile(None, 0)
        if rc != 0:
            raise RuntimeError(f"axon_start_nrt_profile rc={rc}")
        try:
            yield
        finally:
            n = lib.axon_stop_nrt_profile(str(output_dir).encode())
            if n < 0:
                raise RuntimeError(f"axon_stop_nrt_profile rc={n}")

    return _hook


def split_multi_waits(bir_json: bytes) -> bytes:
    d = json.loads(bir_json)
    n_split = 0
    for fn in d.get("functions", []):
        for blk in fn.get("blocks", []):
            insts = blk.get("instructions", [])
            out = []
            for inst in insts:
                si = inst.get("sync_info")
                waits = (si or {}).get("on_wait") or []
                if len(waits) > 1:
                    extra, keep = waits[:-1], waits[-1:]
                    for k, w in enumerate(extra):
                        out.append({
                            "debug": inst.get("debug", 0),
                            "engine": inst["engine"],
                            "ins": [],
                            "outs": [],
                            "name": f"{inst['name']}-ws{k}",
                            "opcode": "NoOp",
                            "sync_info": {"on_update": [], "on_wait": [w]},
                        })
                        n_split += 1
                    si["on_wait"] = keep
                out.append(inst)
            blk["instructions"] = out
    if n_split:
        print(f"bass_infra: split {n_split} extra sync waits into NoOps")
    return json.dumps(d).encode()


def install():
    # 1. antenv.axon_hooks shim
    if "antenv.axon_hooks" not in sys.modules:
        mod = types.ModuleType("antenv.axon_hooks")
        _state = {"hook": _ntff_profile_via_ctypes(_SO_PATH)}
        mod.set_axon_ntff_profile_hook = lambda h: _state.__setitem__("hook", h)
        mod.get_axon_ntff_profile_hook = lambda: _state["hook"]
        sys.modules["antenv.axon_hooks"] = mod
        import antenv
        antenv.axon_hooks = mod

    from concourse import bass_utils, bass2jax

    # 2. upload_artifacts stub
    bass_utils.upload_artifacts = lambda tmpdir: tmpdir

    # 3. wait-splitting compile wrapper
    orig_compile = bass_utils.compile_bir_kernel

    def compile_with_split(bir_json, tmpdir, neff_name="file.neff"):
        return orig_compile(split_multi_waits(bir_json), tmpdir, neff_name=neff_name)

    if getattr(bass2jax.compile_bir_kernel, "__name__", "") != "compile_with_split":
        bass_utils.compile_bir_kernel = compile_with_split
        bass2jax.compile_bir_kernel = compile_with_split


# ======== kernel IR builder ========
from contextlib import ExitStack

import concourse.bass as bass
import concourse.mybir as mybir
import concourse.tile as tile
from concourse.bass import ds, ts
from concourse.masks import make_identity
F32 = mybir.dt.float32
F32R = mybir.dt.float32r
BF = mybir.dt.bfloat16
EXP = mybir.ActivationFunctionType.Exp
LN = mybir.ActivationFunctionType.Ln

B, S, D, H, DK = 4, 2048, 1024, 16, 64
NC = 8          # cores
HL = 2          # heads per core
BS = B * S      # 8192
NQ = S // 512   # q-chunks per batch = 4
NCH = B * NQ    # 16 x-chunks total
NKC = S // 128  # k-chunks per batch = 16
NDC = D // 128  # d_in chunks = 8
POS = BS // NC  # positions per core for out-proj = 1024
STR = 128       # stripe width per (batch, half)
NEG = -8.0e9    # pre-exp mask fill; *0.125 -> -1e9 -> exp == 0
KEEP = 6        # rolling ctx-queue depth (in single-MM entries)


def build(cfg=None):
    cfg = cfg or {}
    nc = bass.Bass("TRN2", target_bir_lowering=False, debug=False, num_devices=NC)

    # all pre-laid per-partition-contiguous by the host wrapper
    xP = nc.dram_tensor("xP", [128, NCH, NDC, 512], BF, kind="ExternalInput")
    wq = nc.dram_tensor("wq", [128, NDC, 128], BF, kind="ExternalInput")
    wk = nc.dram_tensor("wk", [128, NDC, 128], BF, kind="ExternalInput")
    wv = nc.dram_tensor("wv", [128, NDC, 128], BF, kind="ExternalInput")
    wo = nc.dram_tensor("wo", [128, NDC, D], BF, kind="ExternalInput")
    out = nc.dram_tensor("out", [POS, D], F32, kind="ExternalOutput")

    a2a_in = [nc.dram_tensor(f"a2a_in{t}", [NC, 128, STR], BF) for t in range(2 * B)]
    a2a_out = [nc.dram_tensor(f"a2a_out{t}", [NC, 128, STR], BF) for t in range(2 * B)]

    with tile.TileContext(nc) as tc, ExitStack() as ctx:
        const = ctx.enter_context(tc.tile_pool(name="const", bufs=1))
        wpool = ctx.enter_context(tc.tile_pool(name="wpool", bufs=1))
        xpool = ctx.enter_context(tc.tile_pool(name="xpool", bufs=4))
        qk_sb = ctx.enter_context(tc.tile_pool(name="qk_sb", bufs=2))
        vpool = ctx.enter_context(tc.tile_pool(name="vpool", bufs=2))
        et_sb = ctx.enter_context(tc.tile_pool(name="et_sb", bufs=8))
        ep_sb = ctx.enter_context(tc.tile_pool(name="ep_sb", bufs=2))
        ctx_sb = ctx.enter_context(tc.tile_pool(name="ctx_sb", bufs=2))
        os_sb = ctx.enter_context(tc.tile_pool(name="os_sb", bufs=2))
        # PSUM: 3 x 2-bank "sp2" ring + 2 x 1-bank cp ring = 8 banks
        sp_ps = ctx.enter_context(tc.tile_pool(name="sp_ps", bufs=3, space="PSUM"))
        cp_ps = ctx.enter_context(tc.tile_pool(name="cp_ps", bufs=2, space="PSUM"))

        # ---- constants ----
        identf = const.tile([128, 128], F32)
        make_identity(nc, identf[:])
        identb = const.tile([128, 128], BF)
        nc.vector.tensor_copy(identb[:], identf[:])
        onesf = const.tile([128, 16], BF)
        nc.vector.memset(onesf[:], 1.0)
        ones_l = const.tile([1, 64], F32)
        nc.vector.memset(ones_l[:], 1.0)
        ones_lb = const.tile([1, 64], BF)
        nc.vector.tensor_copy(ones_lb[:], ones_l[:])
        # 128-periodic triangle mask: bandm[k, a*128 + y] = 0 if y >= k else NEG
        bandm_f = const.tile([128, 512], F32)
        nc.gpsimd.memset(bandm_f[:], 0.0)
        for a in range(4):
            nc.gpsimd.affine_select(
                out=bandm_f[:, ds(a * 128, 128)], in_=bandm_f[:, ds(a * 128, 128)],
                compare_op=mybir.AluOpType.is_ge, fill=NEG,
                base=0, pattern=[[1, 128]], channel_multiplier=-1,
            )
        bandm = const.tile([128, 512], BF)
        nc.vector.tensor_copy(bandm[:], bandm_f[:])

        # ---- weights ----
        wq_sb = wpool.tile([128, NDC, 128], BF)
        wk_sb = wpool.tile([128, NDC, 128], BF)
        wv_sb = wpool.tile([128, NDC, 128], BF)
        nc.sync.dma_start(wq_sb[:], wq[:])
        nc.sync.dma_start(wk_sb[:], wk[:])
        nc.sync.dma_start(wv_sb[:], wv[:])
        wo_sb = wpool.tile([128, NDC, D], BF)

        def trigger_a2a(t, ctxT, h):
            for j in range(NC):
                nc.sync.dma_start(a2a_in[t][j], ctxT[:, ds(h * 1024 + j * STR, STR)])
            nc.gpsimd.collective_compute(
                "AllToAll", mybir.AluOpType.bypass,
                replica_groups=[list(range(NC))],
                ins=[a2a_in[t][:]], outs=[a2a_out[t][:]],
            )

        def consume_a2a(t, spread=False):
            ctxg = ctx_sb.tile([128, NC, STR], BF, tag="ctxg")
            qs = ([nc.sync, nc.gpsimd, nc.scalar] if spread
                  else [nc.gpsimd])
            for j in range(NC):
                qs[j % len(qs)].dma_start(ctxg[:, j, :], a2a_out[t][j])
            for nn in range(2):
                op = sp_ps.tile([128, 1024], F32, tag="sp2")
                for j in range(NC):
                    nc.tensor.matmul(
                        op[:, 0:512], ctxg[:, j, :], wo_sb[:, j, ts(nn, 512)],
                        start=(j == 0), stop=(j == NC - 1),
                    )
                os_ = os_sb.tile([128, 512], F32, tag="os")
                nc.vector.tensor_copy(os_[:], op[:, 0:512])
                nc.sync.dma_start(out[ds(t * STR, STR), ts(nn, 512)], os_[:])

        def qkv_pieces(b, qt, kt, vaug):
            """Generator: Q/K/V projection for batch b in pieces, so
            attention(b-1) can interleave it as PE filler."""
            nc.vector.tensor_copy(vaug[:, :, 64:65].opt(), onesf[:, 0:NKC])
            nc.vector.tensor_copy(vaug[:, :, 129:130].opt(), onesf[:, 0:NKC])
            for i in range(NQ):
                xt = xpool.tile([128, NDC, 512], BF, tag="x")
                c = b * NQ + i
                if b == 0 and i == 0:
                    # per-j sub-DMAs so the first matmul starts early
                    for j in range(NDC):
                        nc.gpsimd.dma_start(xt[:, j, :], xP[:, c, j, :])
                else:
                    nc.gpsimd.dma_start(xt[:], xP[:, c])
                # piece A: Q and K projections into one 2-bank tile
                qkp = sp_ps.tile([128, 1024], F32, tag="sp2")
                for h, w_sb in ((0, wq_sb), (1, wk_sb)):
                    for j in range(NDC):
                        nc.tensor.matmul(qkp[:, ds(h * 512, 512)], w_sb[:, j, :],
                                         xt[:, j, :],
                                         start=(j == 0), stop=(j == NDC - 1))
                nc.vector.tensor_copy(qt[:, ts(i, 512)], qkp[:, 0:512])
                nc.vector.tensor_copy(kt[:, ts(i, 512)], qkp[:, 512:1024])
                yield
                # piece B: V projection (bank0) + transposes (bank1)
                vv = sp_ps.tile([128, 1024], F32, tag="sp2")
                for j in range(NDC):
                    nc.tensor.matmul(vv[:, 0:512], wv_sb[:, j, :], xt[:, j, :],
                                     start=(j == 0), stop=(j == NDC - 1))
                vs = ep_sb.tile([128, 512], F32, tag="vs")
                nc.vector.tensor_copy(vs[:], vv[:, 0:512])
                for j4 in range(4):
                    ki = i * 4 + j4
                    nc.tensor.transpose(
                        vv[:, ds(512 + j4 * 128, 128)], vs[:, ts(j4, 128)],
                        identf[:])
                    nc.vector.tensor_copy(
                        vaug[:, ki].rearrange("p (g c) -> p g c", g=2)[:, :, 0:64],
                        vv[:, ds(512 + j4 * 128, 128)].rearrange(
                            "p (g c) -> p g c", g=2),
                    )
                yield

        # ---- rolling cross-stage pipelines ----
        # ctxq: (stage_seq, closure) entries — ctx accumulation groups and
        # the trailing phase1; kept at depth<=KEEP so the in-order PE queue
        # always has next-stage score work between a ctx MM and the exp it
        # depends on. Entries older than the previous stage are force-
        # drained at each stage start so tile-ring reuse never outruns the
        # emission of the previous user's readers.
        ctxq = []

        def pump_ctx(keep):
            while len(ctxq) > keep:
                ctxq.pop(0)[1]()

        def pump_until_stage(min_stage):
            while ctxq and ctxq[0][0] < min_stage:
                ctxq.pop(0)[1]()

        phase2_q = []

        def run_phase2():
            if not phase2_q:
                return
            cp, hh, qi, ctxT, rr, cb = phase2_q.pop(0)
            bcp = sp_ps.tile([128, 1024], F32, tag="sp2")
            nc.tensor.matmul(bcp[0:64, 0:512], ones_lb[:], rr[:],
                             start=True, stop=True)
            bcs = ep_sb.tile([64, 512], BF, tag="bcs")
            nc.vector.tensor_copy(bcs[:], bcp[0:64, 0:512])
            nc.vector.tensor_mul(
                ctxT[ds(64 * hh, 64), ts(qi, 512)], cp[0:64, :], bcs[:])
            if cb is not None:
                cb()

        def phase1(cp, hh, qi, ctxT, cb):
            # 1/denom = exp(-ln(denom)) on ACT; trails in the ACT queue
            lg = ep_sb.tile([1, 512], F32, tag="lg")
            nc.scalar.activation(lg[:], cp[64:65, :], LN)
            rr = ep_sb.tile([1, 512], BF, tag="rr")
            nc.scalar.activation(rr[:], lg[:], EXP, scale=-1.0)
            phase2_q.append((cp, hh, qi, ctxT, rr, cb))

        def attn_stage(sseq, b, qi, hh, st, fill, cb):
            q0 = qi * 512
            pump_until_stage(sseq - 1)
            qt, kt, vaug, ctxT = st["qt"], st["kt"], st["vaug"], st["ctxT"]

            def pump_fill(n):
                for _ in range(n):
                    if fill is not None:
                        try:
                            next(fill)
                        except StopIteration:
                            break

            cp = cp_ps.tile([65, 512], F32, tag="cp")
            nctx = [0]
            NTOT = 4 * qi + 7

            def ctx_mm(out_ap, lhsT, rhs):
                i = nctx[0]
                nc.tensor.matmul(out_ap, lhsT, rhs,
                                 start=(i == 0), stop=(i == NTOT - 1),
                                 skip_group_check=True)
                nctx[0] += 1

            krow = kt[ds(64 * hh, 64), :]
            qrow = qt[ds(64 * hh, 64), ds(q0, 512)]

            # -- full (strictly below-diagonal) k-chunk pairs --
            for kp in range(2 * qi):
                sp2 = sp_ps.tile([128, 1024], F32, tag="sp2")
                for h in range(2):
                    ki = 2 * kp + h
                    nc.tensor.matmul(
                        sp2[:, ds(h * 512, 512)],
                        krow[:, ts(ki, 128)].opt(), qrow.opt(),
                        start=True, stop=True)
                et2 = et_sb.tile([128, 1024], BF, tag="et")
                nc.scalar.activation(et2[:], sp2[:], EXP, scale=0.125)

                def emit(kp=kp, et2=et2, cp=cp, ctx_mm=ctx_mm, hh=hh):
                    for h in range(2):
                        ctx_mm(cp[:], vaug[:, 2 * kp + h, ds(65 * hh, 65)],
                               et2[:, ds(h * 512, 512)])
                ctxq.append((sseq, emit))
                pump_ctx(KEEP)
                if kp == 0:
                    run_phase2()
                if kp == 1:
                    pump_fill(1)

            # -- diagonal chunks ki = 4*qi + a, a in 0..3 --
            # strictly-valid partials packed in spD: bank0 = a0 (384 cols) +
            # a2 (128); bank1 = a1 (256). The 128-wide diagonal band of all
            # four goes in spE bank0 on top of the periodic triangle mask.
            spD = sp_ps.tile([128, 1024], F32, tag="sp2")
            nc.tensor.matmul(
                spD[:, 0:384], krow[:, ts(4 * qi, 128)].opt(),
                qt[ds(64 * hh, 64), ds(q0 + 128, 384)].opt(),
                start=True, stop=False, skip_group_check=True)
            nc.tensor.matmul(
                spD[:, 384:512], krow[:, ts(4 * qi + 2, 128)].opt(),
                qt[ds(64 * hh, 64), ds(q0 + 384, 128)].opt(),
                start=False, stop=True, skip_group_check=True)
            nc.tensor.matmul(
                spD[:, 512:768], krow[:, ts(4 * qi + 1, 128)].opt(),
                qt[ds(64 * hh, 64), ds(q0 + 256, 256)].opt(),
                start=True, stop=True, skip_group_check=True)
            etD = et_sb.tile([128, 1024], BF, tag="et")
            nc.scalar.activation(etD[:, 0:768], spD[:, 0:768], EXP, scale=0.125)
            if qi == 0:
                run_phase2()

            spE = sp_ps.tile([128, 1024], F32, tag="sp2")
            nc.tensor.matmul(spE[:, 0:512], identb[:], bandm[:],
                             start=True, stop=False, skip_group_check=True)
            for a in range(4):
                nc.tensor.matmul(
                    spE[:, ts(a, 128)],
                    krow[:, ts(4 * qi + a, 128)].opt(),
                    qt[ds(64 * hh, 64), ds(q0 + a * 128, 128)].opt(),
                    start=False, stop=(a == 3), skip_group_check=True)
            etE = et_sb.tile([128, 1024], BF, tag="et")
            nc.scalar.activation(etE[:, 0:512], spE[:, 0:512], EXP, scale=0.125)

            def emit_diag(cp=cp, etD=etD, etE=etE, qi=qi, hh=hh, ctx_mm=ctx_mm,
                          ctxT=ctxT, cb=cb):
                ctx_mm(cp[:, ds(128, 384)], vaug[:, 4 * qi, ds(65 * hh, 65)],
                       etD[:, 0:384])
                ctx_mm(cp[:, ds(384, 128)], vaug[:, 4 * qi + 2, ds(65 * hh, 65)],
                       etD[:, 384:512])
                ctx_mm(cp[:, ds(256, 256)], vaug[:, 4 * qi + 1, ds(65 * hh, 65)],
                       etD[:, 512:768])
                for a in range(4):
                    ctx_mm(cp[:, ds(a * 128, 128)],
                           vaug[:, 4 * qi + a, ds(65 * hh, 65)],
                           etE[:, ts(a, 128)])
                phase1(cp, hh, qi, ctxT, cb)
            ctxq.append((sseq, emit_diag))
            pump_ctx(KEEP)
            pump_fill(1)

        # warmup collective: absorbs the one-time CC init cost during QKV(0)
        wu_in = nc.dram_tensor("wu_in", [NC, 128, 4], F32)
        wu_out = nc.dram_tensor("wu_out", [NC, 128, 4], F32)
        wu = const.tile([128, NC * 4], F32)
        nc.vector.memset(wu[:], 0.0)
        nc.sync.dma_start(wu_in[:], wu[:].rearrange("p (j n) -> j p n", j=NC))
        nc.gpsimd.collective_compute(
            "AllToAll", mybir.AluOpType.bypass,
            replica_groups=[list(range(NC))],
            ins=[wu_in[:]], outs=[wu_out[:]],
        )

        pending = []  # triggered but not yet consumed A2A ids

        def consume_oldest(keep, spread=False):
            while len(pending) > keep:
                consume_a2a(pending.pop(0), spread=spread)

        def make_state(b):
            st = {
                "ctxT": ctx_sb.tile([128, S], BF, tag="ctx", name=f"ctxT{b}"),
                "qt": qk_sb.tile([128, S], BF, tag="qt", name=f"qt{b}"),
                "kt": qk_sb.tile([128, S], BF, tag="kt", name=f"kt{b}"),
                "vaug": vpool.tile([128, NKC, 130], BF, name=f"vaug{b}"),
            }
            st["gen"] = qkv_pieces(b, st["qt"], st["kt"], st["vaug"])
            return st

        st = make_state(0)
        for _ in st["gen"]:
            pass

        for b in range(B):
            nxt = make_state(b + 1) if b + 1 < B else None
            fill = nxt["gen"] if nxt else None

            def mid_cb(b=b, ctxT=st["ctxT"]):
                trigger_a2a(2 * b, ctxT, 0)
                if b == 0:
                    nc.sync.dma_start(wo_sb[:], wo[:])
                pending.append(2 * b)
                consume_oldest(2)

            def end_cb(b=b, ctxT=st["ctxT"]):
                trigger_a2a(2 * b + 1, ctxT, 1)
                pending.append(2 * b + 1)
                consume_oldest(2)

            for qi in range(NQ):
                for hh in range(HL):
                    cb = None
                    if hh == 1 and qi == 1:
                        cb = mid_cb
                    elif hh == 1 and qi == 3:
                        cb = end_cb
                    attn_stage(b * 8 + qi * 2 + hh, b, qi, hh, st, fill, cb)
            if fill is not None:
                for _ in fill:
                    pass
            st = nxt
        pump_ctx(0)
        while phase2_q:
            run_phase2()
        consume_oldest(1)
        consume_oldest(0, spread=True)

    return nc


# ======== host-side wrapper ========
_CACHE = {}


def _get_program():
    if "nc" not in _CACHE:
        install()
        _CACHE["nc"] = build()
    return _CACHE["nc"]


def _run(inputs, trace=False):
    import ml_dtypes
    from concourse.bass_utils import run_bass_kernel_spmd

    BFNP = ml_dtypes.bfloat16
    x = np.asarray(inputs["x"], dtype=np.float32)
    WQ = np.asarray(inputs["WQ"], dtype=np.float32)
    WK = np.asarray(inputs["WK"], dtype=np.float32)
    WV = np.asarray(inputs["WV"], dtype=np.float32)
    WO = np.asarray(inputs["WO"], dtype=np.float32)

    # per-partition-contiguous layouts (low DMA descriptor count)
    xTh = x.reshape(BS, D).T                       # [D, BS]
    xPh = np.ascontiguousarray(
        xTh.reshape(NDC, 128, NCH, 512).transpose(1, 2, 0, 3).astype(BFNP))
    woP = np.ascontiguousarray(
        WO.T.reshape(NDC, 128, D).transpose(1, 0, 2).astype(BFNP))

    def wprep(W, c):
        sl = slice(c * 128, (c + 1) * 128)
        return np.ascontiguousarray(
            W[sl, :].T.reshape(NDC, 128, 128).transpose(1, 0, 2).astype(BFNP))

    in_maps = []
    for c in range(NC):
        in_maps.append({
            "xP": xPh,
            "wq": wprep(WQ, c),
            "wk": wprep(WK, c),
            "wv": wprep(WV, c),
            "wo": woP,
        })

    nc_prog = _get_program()
    res = run_bass_kernel_spmd(nc_prog, in_maps, list(range(NC)), trace=trace)

    actual = np.zeros((BS, D), dtype=np.float32)
    for c in range(NC):
        oc = res.results[c]["out"]
        for b in range(B):
            for h in range(2):
                t = 2 * b + h
                r0 = b * S + h * 1024 + c * STR
                actual[r0:r0 + STR] = oc[(t * STR):(t + 1) * STR]
    return actual.reshape(x.shape), res


def kernel(**inputs):
    out, _ = _run(inputs, trace=False)
    return out


# revision 15
# speedup vs baseline: 1.0411x; 1.0411x over previous
"""Self-contained Trainium2 kernel for nn_MultiHeadAttention_91070486544496.

B=4, S=2048, D=1024, H=16 causal MHA. 8-core SPMD: head-parallel
QKV+attention (2 heads/core), mid-attention AllToAll reshard, then
position-parallel output projection.

v4: bf16 matmul pipeline (fp32 streams at 2 cyc/col on the PE array);
causal diagonal via partial-width score MMs plus a 128-periodic
triangle-mask band tile (one start=True per PSUM bank per group —
start poisons the whole 2KB bank); paired [128,1024] score tiles to
amortize ACT op overhead; a global rolling ctx-MM queue that crosses
stage boundaries so the in-order PE queue never waits on the current
stage's last exp; softmax normalization (ln/exp on ACT) phase-split
across stages; host pre-laid per-partition-contiguous DRAM inputs;
DMA dispatch spread across sync+gpsimd queues; next-batch QKV
interleaved into the attention stream as PE filler.
"""
import sys

for _p in ("/opt/trn_rl_repo", "/root/.axon_site/_ro/trn_rl_repo"):
    if _p not in sys.path:
        sys.path.append(_p)

import numpy as np

# ======== runtime infra (axon NTFF hook, BIR wait splitter) ========

import contextlib
import ctypes
import json
import sys
import types

_SO_PATH = "/opt/axon/libaxon_pjrt.so"


def _ntff_profile_via_ctypes(so_path):
    lib = ctypes.CDLL(so_path)
    if not hasattr(lib, "axon_start_nrt_profile"):
        return None
    lib.axon_start_nrt_profile.argtypes = [
        ctypes.POINTER(ctypes.c_int64),
        ctypes.c_size_t,
    ]
    lib.axon_start_nrt_profile.restype = ctypes.c_int64
    lib.axon_stop_nrt_profile.argtypes = [ctypes.c_char_p]
    lib.axon_stop_nrt_profile.restype = ctypes.c_int64

    @contextlib.contextmanager
    def _hook(output_dir, device_ids):
        import jax
        jax.devices()
        if device_ids:
            ids = (ctypes.c_int64 * len(device_ids))(*device_ids)
            rc = lib.axon_start_nrt_profile(ids, len(device_ids))
        else:
            rc = lib.axon_start_nrt_profile(None, 0)
        if rc != 0:
            raise RuntimeError(f"axon_start_nrt_profile rc={rc}")
        try:
            yield
        finally:
            n = lib.axon_stop_nrt_profile(str(output_dir).encode())
            if n < 0:
                raise RuntimeError(f"axon_stop_nrt_profile rc={n}")

    return _hook


def split_multi_waits(bir_json: bytes) -> bytes:
    d = json.loads(bir_json)
    n_split = 0
    for fn in d.get("functions", []):
        for blk in fn.get("blocks", []):
            insts = blk.get("instructions", [])
            out = []
            for inst in insts:
                si = inst.get("sync_info")
                waits = (si or {}).get("on_wait") or []
                if len(waits) > 1:
                    extra, keep = waits[:-1], waits[-1:]
                    for k, w in enumerate(extra):
                        out.append({
                            "debug": inst.get("debug", 0),
                            "engine": inst["engine"],
                            "ins": [],
                            "outs": [],
                            "name": f"{inst['name']}-ws{k}",
                            "opcode": "NoOp",
                            "sync_info": {"on_update": [], "on_wait": [w]},
                        })
                        n_split += 1
                    si["on_wait"] = keep
                out.append(inst)
            blk["instructions"] = out
    if n_split:
        print(f"bass_infra: split {n_split} extra sync waits into NoOps")
    return json.dumps(d).encode()


def install():
    # 1. antenv.axon_hooks shim
    if "antenv.axon_hooks" not in sys.modules:
        mod = types.ModuleType("antenv.axon_hooks")
        _state = {"hook": _ntff_profile_via_ctypes(_SO_PATH)}
        mod.set_axon_ntff_profile_hook = lambda h: _state.__setitem__("hook", h)
        mod.get_axon_ntff_profile_hook = lambda: _state["hook"]
        sys.modules["antenv.axon_hooks"] = mod
        import antenv
        antenv.axon_hooks = mod

    from concourse import bass_utils, bass2jax

    # 2. upload_artifacts stub
    bass_utils.upload_artifacts = lambda tmpdir: tmpdir

    # 3. wait-splitting compile wrapper
    orig_compile = bass_utils.compile_bir_kernel

    def compile_with_split(bir_json, tmpdir, neff_name="file.neff"):
        return orig_compile(split_multi_waits(bir_json), tmpdir, neff_name=neff_name)

    if getattr(bass2jax.compile_bir_kernel, "__name__", "") != "compile_with_split":
        bass_utils.compile_bir_kernel = compile_with_split
        bass2jax.compile_bir_kernel = compile_with_split


# ======== kernel IR builder ========
from contextlib import ExitStack

import concourse.bass as bass
import concourse.mybir as mybir
import concourse.tile as tile
from concourse.bass import ds, ts
from concourse.masks import make_identity
F32 = mybir.dt.float32
F32R = mybir.dt.float32r
BF = mybir.dt.bfloat16
EXP = mybir.ActivationFunctionType.Exp
LN = mybir.ActivationFunctionType.Ln

B, S, D, H, DK = 4, 2048, 1024, 16, 64
NC = 8          # cores
HL = 2          # heads per core
BS = B * S      # 8192
NQ = S // 512   # q-chunks per batch = 4
NCH = B * NQ    # 16 x-chunks total
NKC = S // 128  # k-chunks per batch = 16
NDC = D // 128  # d_in chunks = 8
POS = BS // NC  # positions per core for out-proj = 1024
STR = 128       # stripe width per (batch, half)
NEG = -8.0e9    # pre-exp mask fill; *0.125 -> -1e9 -> exp == 0
KEEP = 6        # rolling ctx-queue depth (in single-MM entries)


def build(cfg=None):
    cfg = cfg or {}
    nc = bass.Bass("TRN2", target_bir_lowering=False, debug=False, num_devices=NC)

    # all pre-laid per-partition-contiguous by the host wrapper
    xP = nc.dram_tensor("xP", [128, NCH, NDC, 512], BF, kind="ExternalInput")
    wq = nc.dram_tensor("wq", [128, NDC, 128], BF, kind="ExternalInput")
    wk = nc.dram_tensor("wk", [128, NDC, 128], BF, kind="ExternalInput")
    wv = nc.dram_tensor("wv", [128, NDC, 128], BF, kind="ExternalInput")
    wo = nc.dram_tensor("wo", [128, NDC, D], BF, kind="ExternalInput")
    out = nc.dram_tensor("out", [POS, D], F32, kind="ExternalOutput")

    a2a_in = [nc.dram_tensor(f"a2a_in{t}", [NC, 128, STR], BF) for t in range(2 * B)]
    a2a_out = [nc.dram_tensor(f"a2a_out{t}", [NC, 128, STR], BF) for t in range(2 * B)]

    with tile.TileContext(nc) as tc, ExitStack() as ctx:
        const = ctx.enter_context(tc.tile_pool(name="const", bufs=1))
        wpool = ctx.enter_context(tc.tile_pool(name="wpool", bufs=1))
        xpool = ctx.enter_context(tc.tile_pool(name="xpool", bufs=4))
        qk_sb = ctx.enter_context(tc.tile_pool(name="qk_sb", bufs=2))
        vpool = ctx.enter_context(tc.tile_pool(name="vpool", bufs=2))
        et_sb = ctx.enter_context(tc.tile_pool(name="et_sb", bufs=12))
        ep_sb = ctx.enter_context(tc.tile_pool(name="ep_sb", bufs=2))
        ctx_sb = ctx.enter_context(tc.tile_pool(name="ctx_sb", bufs=2))
        os_sb = ctx.enter_context(tc.tile_pool(name="os_sb", bufs=2))
        # PSUM: 3 x 2-bank "sp2" ring + 2 x 1-bank cp ring = 8 banks
        sp_ps = ctx.enter_context(tc.tile_pool(name="sp_ps", bufs=3, space="PSUM"))
        cp_ps = ctx.enter_context(tc.tile_pool(name="cp_ps", bufs=2, space="PSUM"))

        # ---- constants ----
        identf = const.tile([128, 128], F32)
        make_identity(nc, identf[:])
        identb = const.tile([128, 128], BF)
        nc.vector.tensor_copy(identb[:], identf[:])
        onesf = const.tile([128, 16], BF)
        nc.vector.memset(onesf[:], 1.0)
        ones_l = const.tile([1, 64], F32)
        nc.vector.memset(ones_l[:], 1.0)
        ones_lb = const.tile([1, 64], BF)
        nc.vector.tensor_copy(ones_lb[:], ones_l[:])
        # 128-periodic triangle mask: bandm[k, a*128 + y] = 0 if y >= k else NEG
        bandm_f = const.tile([128, 512], F32)
        nc.gpsimd.memset(bandm_f[:], 0.0)
        for a in range(4):
            nc.gpsimd.affine_select(
                out=bandm_f[:, ds(a * 128, 128)], in_=bandm_f[:, ds(a * 128, 128)],
                compare_op=mybir.AluOpType.is_ge, fill=NEG,
                base=0, pattern=[[1, 128]], channel_multiplier=-1,
            )
        bandm = const.tile([128, 512], BF)
        nc.vector.tensor_copy(bandm[:], bandm_f[:])
        # PE clock warmup while input DMAs are in flight
        wup = sp_ps.tile([128, 1024], F32, tag="sp2")
        for _ in range(24):
            nc.tensor.matmul(wup[:, 0:512], identb[:], bandm[:],
                             start=True, stop=True)

        # ---- weights ----
        wq_sb = wpool.tile([128, NDC, 128], BF)
        wk_sb = wpool.tile([128, NDC, 128], BF)
        wv_sb = wpool.tile([128, NDC, 128], BF)
        nc.sync.dma_start(wq_sb[:], wq[:])
        nc.sync.dma_start(wk_sb[:], wk[:])
        nc.sync.dma_start(wv_sb[:], wv[:])
        wo_sb = wpool.tile([128, NDC, D], BF)

        def trigger_a2a(t, ctxT, h, spread=False):
            qs = ([nc.sync, nc.gpsimd, nc.scalar] if spread else [nc.sync])
            for j in range(NC):
                qs[j % len(qs)].dma_start(
                    a2a_in[t][j], ctxT[:, ds(h * 1024 + j * STR, STR)])
            nc.gpsimd.collective_compute(
                "AllToAll", mybir.AluOpType.bypass,
                replica_groups=[list(range(NC))],
                ins=[a2a_in[t][:]], outs=[a2a_out[t][:]],
            )

        def consume_a2a(t, spread=False):
            ctxg = ctx_sb.tile([128, NC, STR], BF, tag="ctxg")
            qs = ([nc.sync, nc.gpsimd, nc.scalar] if spread
                  else [nc.gpsimd])
            for j in range(NC):
                qs[j % len(qs)].dma_start(ctxg[:, j, :], a2a_out[t][j])
            for nn in range(2):
                op = sp_ps.tile([128, 1024], F32, tag="sp2")
                for j in range(NC):
                    nc.tensor.matmul(
                        op[:, 0:512], ctxg[:, j, :], wo_sb[:, j, ts(nn, 512)],
                        start=(j == 0), stop=(j == NC - 1),
                    )
                os_ = os_sb.tile([128, 512], F32, tag="os")
                nc.vector.tensor_copy(os_[:], op[:, 0:512])
                nc.sync.dma_start(out[ds(t * STR, STR), ts(nn, 512)], os_[:])

        def qkv_pieces(b, qt, kt, vaug):
            """Generator: Q/K/V projection for batch b in pieces, so
            attention(b-1) can interleave it as PE filler."""
            nc.vector.tensor_copy(vaug[:, :, 64:65].opt(), onesf[:, 0:NKC])
            nc.vector.tensor_copy(vaug[:, :, 129:130].opt(), onesf[:, 0:NKC])
            for i in range(NQ):
                xt = xpool.tile([128, NDC, 512], BF, tag="x")
                c = b * NQ + i
                if b == 0 and i == 0:
                    # per-j sub-DMAs so the first matmul starts early
                    for j in range(NDC):
                        nc.gpsimd.dma_start(xt[:, j, :], xP[:, c, j, :])
                else:
                    nc.gpsimd.dma_start(xt[:], xP[:, c])
                # piece A: Q and K projections into one 2-bank tile
                qkp = sp_ps.tile([128, 1024], F32, tag="sp2")
                for h, w_sb in ((0, wq_sb), (1, wk_sb)):
                    for j in range(NDC):
                        nc.tensor.matmul(qkp[:, ds(h * 512, 512)], w_sb[:, j, :],
                                         xt[:, j, :],
                                         start=(j == 0), stop=(j == NDC - 1))
                nc.vector.tensor_copy(qt[:, ts(i, 512)], qkp[:, 0:512])
                nc.vector.tensor_copy(kt[:, ts(i, 512)], qkp[:, 512:1024])
                yield
                # piece B: V projection (bank0) + transposes (bank1)
                vv = sp_ps.tile([128, 1024], F32, tag="sp2")
                for j in range(NDC):
                    nc.tensor.matmul(vv[:, 0:512], wv_sb[:, j, :], xt[:, j, :],
                                     start=(j == 0), stop=(j == NDC - 1))
                vs = ep_sb.tile([128, 512], F32, tag="vs")
                nc.vector.tensor_copy(vs[:], vv[:, 0:512])
                for j4 in range(4):
                    ki = i * 4 + j4
                    nc.tensor.transpose(
                        vv[:, ds(512 + j4 * 128, 128)], vs[:, ts(j4, 128)],
                        identf[:])
                    nc.vector.tensor_copy(
                        vaug[:, ki].rearrange("p (g c) -> p g c", g=2)[:, :, 0:64],
                        vv[:, ds(512 + j4 * 128, 128)].rearrange(
                            "p (g c) -> p g c", g=2),
                    )
                yield

        # ---- rolling cross-stage pipelines ----
        # ctxq: (stage_seq, closure) entries — ctx accumulation groups and
        # the trailing phase1; kept at depth<=KEEP so the in-order PE queue
        # always has next-stage score work between a ctx MM and the exp it
        # depends on. Entries older than the previous stage are force-
        # drained at each stage start so tile-ring reuse never outruns the
        # emission of the previous user's readers.
        ctxq = []

        def pump_ctx(keep):
            while len(ctxq) > keep:
                ctxq.pop(0)[1]()

        def pump_until_stage(min_stage):
            while ctxq and ctxq[0][0] < min_stage:
                ctxq.pop(0)[1]()

        phase2_q = []

        def run_phase2():
            if not phase2_q:
                return
            cp, hh, qi, ctxT, rr, cb = phase2_q.pop(0)
            bcp = sp_ps.tile([128, 1024], F32, tag="sp2")
            nc.tensor.matmul(bcp[0:64, 0:512], ones_lb[:], rr[:],
                             start=True, stop=True)
            bcs = ep_sb.tile([64, 512], BF, tag="bcs")
            nc.vector.tensor_copy(bcs[:], bcp[0:64, 0:512])
            nc.vector.tensor_mul(
                ctxT[ds(64 * hh, 64), ts(qi, 512)], cp[0:64, :], bcs[:])
            if cb is not None:
                cb()

        def phase1(cp, hh, qi, ctxT, cb):
            # 1/denom = exp(-ln(denom)) on ACT; trails in the ACT queue
            lg = ep_sb.tile([1, 512], F32, tag="lg")
            nc.scalar.activation(lg[:], cp[64:65, :], LN)
            rr = ep_sb.tile([1, 512], BF, tag="rr")
            nc.scalar.activation(rr[:], lg[:], EXP, scale=-1.0)
            phase2_q.append((cp, hh, qi, ctxT, rr, cb))

        def attn_stage(sseq, b, qi, hh, st, fill, cb):
            q0 = qi * 512
            pump_until_stage(sseq - 1)
            qt, kt, vaug, ctxT = st["qt"], st["kt"], st["vaug"], st["ctxT"]

            def pump_fill(n):
                for _ in range(n):
                    if fill is not None:
                        try:
                            next(fill)
                        except StopIteration:
                            break

            cp = cp_ps.tile([65, 512], F32, tag="cp")
            nctx = [0]
            NTOT = 4 * qi + 7

            def ctx_mm(out_ap, lhsT, rhs):
                i = nctx[0]
                nc.tensor.matmul(out_ap, lhsT, rhs,
                                 start=(i == 0), stop=(i == NTOT - 1),
                                 skip_group_check=True)
                nctx[0] += 1

            krow = kt[ds(64 * hh, 64), :]
            qrow = qt[ds(64 * hh, 64), ds(q0, 512)]

            # -- full (strictly below-diagonal) k-chunk pairs --
            for kp in range(2 * qi):
                sp2 = sp_ps.tile([128, 1024], F32, tag="sp2")
                for h in range(2):
                    ki = 2 * kp + h
                    nc.tensor.matmul(
                        sp2[:, ds(h * 512, 512)],
                        krow[:, ts(ki, 128)].opt(), qrow.opt(),
                        start=True, stop=True)
                et2 = et_sb.tile([128, 1024], BF, tag="et")
                nc.scalar.activation(et2[:], sp2[:], EXP, scale=0.125)

                def emit(kp=kp, et2=et2, cp=cp, ctx_mm=ctx_mm, hh=hh):
                    for h in range(2):
                        ctx_mm(cp[:], vaug[:, 2 * kp + h, ds(65 * hh, 65)],
                               et2[:, ds(h * 512, 512)])
                ctxq.append((sseq, emit))
                pump_ctx(KEEP)
                if kp == 0:
                    run_phase2()
                if kp == 1:
                    pump_fill(1)

            # -- diagonal chunks ki = 4*qi + a, a in 0..3 --
            # strictly-valid partials packed in spD: bank0 = a0 (384 cols) +
            # a2 (128); bank1 = a1 (256). The 128-wide diagonal band of all
            # four goes in spE bank0 on top of the periodic triangle mask.
            spD = sp_ps.tile([128, 1024], F32, tag="sp2")
            nc.tensor.matmul(
                spD[:, 0:384], krow[:, ts(4 * qi, 128)].opt(),
                qt[ds(64 * hh, 64), ds(q0 + 128, 384)].opt(),
                start=True, stop=False, skip_group_check=True)
            nc.tensor.matmul(
                spD[:, 384:512], krow[:, ts(4 * qi + 2, 128)].opt(),
                qt[ds(64 * hh, 64), ds(q0 + 384, 128)].opt(),
                start=False, stop=True, skip_group_check=True)
            nc.tensor.matmul(
                spD[:, 512:768], krow[:, ts(4 * qi + 1, 128)].opt(),
                qt[ds(64 * hh, 64), ds(q0 + 256, 256)].opt(),
                start=True, stop=True, skip_group_check=True)
            etD = et_sb.tile([128, 1024], BF, tag="et")
            nc.scalar.activation(etD[:, 0:768], spD[:, 0:768], EXP, scale=0.125)
            if qi == 0:
                run_phase2()

            spE = sp_ps.tile([128, 1024], F32, tag="sp2")
            nc.tensor.matmul(spE[:, 0:512], identb[:], bandm[:],
                             start=True, stop=False, skip_group_check=True)
            for a in range(4):
                nc.tensor.matmul(
                    spE[:, ts(a, 128)],
                    krow[:, ts(4 * qi + a, 128)].opt(),
                    qt[ds(64 * hh, 64), ds(q0 + a * 128, 128)].opt(),
                    start=False, stop=(a == 3), skip_group_check=True)
            etE = et_sb.tile([128, 1024], BF, tag="et")
            nc.scalar.activation(etE[:, 0:512], spE[:, 0:512], EXP, scale=0.125)

            def emit_diag(cp=cp, etD=etD, etE=etE, qi=qi, hh=hh, ctx_mm=ctx_mm,
                          ctxT=ctxT, cb=cb):
                ctx_mm(cp[:, ds(128, 384)], vaug[:, 4 * qi, ds(65 * hh, 65)],
                       etD[:, 0:384])
                ctx_mm(cp[:, ds(384, 128)], vaug[:, 4 * qi + 2, ds(65 * hh, 65)],
                       etD[:, 384:512])
                ctx_mm(cp[:, ds(256, 256)], vaug[:, 4 * qi + 1, ds(65 * hh, 65)],
                       etD[:, 512:768])
                for a in range(4):
                    ctx_mm(cp[:, ds(a * 128, 128)],
                           vaug[:, 4 * qi + a, ds(65 * hh, 65)],
                           etE[:, ts(a, 128)])
                phase1(cp, hh, qi, ctxT, cb)
            ctxq.append((sseq, emit_diag))
            pump_ctx(KEEP)
            pump_fill(1)

        # warmup collective: absorbs the one-time CC init cost during QKV(0)
        wu_in = nc.dram_tensor("wu_in", [NC, 128, 4], F32)
        wu_out = nc.dram_tensor("wu_out", [NC, 128, 4], F32)
        wu = const.tile([128, NC * 4], F32)
        nc.vector.memset(wu[:], 0.0)
        nc.sync.dma_start(wu_in[:], wu[:].rearrange("p (j n) -> j p n", j=NC))
        nc.gpsimd.collective_compute(
            "AllToAll", mybir.AluOpType.bypass,
            replica_groups=[list(range(NC))],
            ins=[wu_in[:]], outs=[wu_out[:]],
        )

        pending = []  # triggered but not yet consumed A2A ids

        def consume_oldest(keep, spread=False):
            while len(pending) > keep:
                consume_a2a(pending.pop(0), spread=spread)

        def make_state(b):
            st = {
                "ctxT": ctx_sb.tile([128, S], BF, tag="ctx", name=f"ctxT{b}"),
                "qt": qk_sb.tile([128, S], BF, tag="qt", name=f"qt{b}"),
                "kt": qk_sb.tile([128, S], BF, tag="kt", name=f"kt{b}"),
                "vaug": vpool.tile([128, NKC, 130], BF, name=f"vaug{b}"),
            }
            st["gen"] = qkv_pieces(b, st["qt"], st["kt"], st["vaug"])
            return st

        import itertools

        st = make_state(0)
        for _ in range(4):
            next(st["gen"])  # pre-drain chunks 0-1; rest streams into b=0

        for b in range(B):
            nxt = make_state(b + 1) if b + 1 < B else None
            if b == 0:
                fill = (itertools.chain(st["gen"], nxt["gen"])
                        if nxt else st["gen"])
            else:
                fill = nxt["gen"] if nxt else None

            def mid_cb(b=b, ctxT=st["ctxT"]):
                trigger_a2a(2 * b, ctxT, 0)
                if b == 0:
                    nc.sync.dma_start(wo_sb[:], wo[:])
                pending.append(2 * b)
                consume_oldest(2)

            def end_cb(b=b, ctxT=st["ctxT"]):
                trigger_a2a(2 * b + 1, ctxT, 1, spread=(b == B - 1))
                pending.append(2 * b + 1)
                consume_oldest(2)

            for qi in range(NQ):
                for hh in range(HL):
                    cb = None
                    if hh == 1 and qi == 1:
                        cb = mid_cb
                    elif hh == 1 and qi == 3:
                        cb = end_cb
                    attn_stage(b * 8 + qi * 2 + hh, b, qi, hh, st, fill, cb)
            if fill is not None:
                for _ in fill:
                    pass
            st = nxt
        pump_ctx(0)
        while phase2_q:
            run_phase2()
        consume_oldest(1)
        consume_oldest(0, spread=True)

    return nc


# ======== host-side wrapper ========
_CACHE = {}


def _get_program():
    if "nc" not in _CACHE:
        install()
        _CACHE["nc"] = build()
    return _CACHE["nc"]


def _run(inputs, trace=False):
    import ml_dtypes
    from concourse.bass_utils import run_bass_kernel_spmd

    BFNP = ml_dtypes.bfloat16
    x = np.asarray(inputs["x"], dtype=np.float32)
    WQ = np.asarray(inputs["WQ"], dtype=np.float32)
    WK = np.asarray(inputs["WK"], dtype=np.float32)
    WV = np.asarray(inputs["WV"], dtype=np.float32)
    WO = np.asarray(inputs["WO"], dtype=np.float32)

    # per-partition-contiguous layouts (low DMA descriptor count)
    xTh = x.reshape(BS, D).T                       # [D, BS]
    xPh = np.ascontiguousarray(
        xTh.reshape(NDC, 128, NCH, 512).transpose(1, 2, 0, 3).astype(BFNP))
    woP = np.ascontiguousarray(
        WO.T.reshape(NDC, 128, D).transpose(1, 0, 2).astype(BFNP))

    def wprep(W, c):
        sl = slice(c * 128, (c + 1) * 128)
        return np.ascontiguousarray(
            W[sl, :].T.reshape(NDC, 128, 128).transpose(1, 0, 2).astype(BFNP))

    in_maps = []
    for c in range(NC):
        in_maps.append({
            "xP": xPh,
            "wq": wprep(WQ, c),
            "wk": wprep(WK, c),
            "wv": wprep(WV, c),
            "wo": woP,
        })

    nc_prog = _get_program()
    res = run_bass_kernel_spmd(nc_prog, in_maps, list(range(NC)), trace=trace)

    actual = np.zeros((BS, D), dtype=np.float32)
    for c in range(NC):
        oc = res.results[c]["out"]
        for b in range(B):
            for h in range(2):
                t = 2 * b + h
                r0 = b * S + h * 1024 + c * STR
                actual[r0:r0 + STR] = oc[(t * STR):(t + 1) * STR]
    return actual.reshape(x.shape), res


def kernel(**inputs):
    out, _ = _run(inputs, trace=False)
    return out
